# revision 12
# baseline (speedup 1.0000x reference)
"""MoE experts kernel for Trainium2 (Bass/Tile), expert-parallel across 8 NeuronCores.

Problem: nn_CompressedMoeExperts — T=2048 tokens, D=1024, FF=1536, E=8 experts,
top-k=2.  out[t] = sum_e combine[e,t] * (silu(h[t] @ Wg[e].T) * (h[t] @ Wu[e].T)) @ Wd[e].T

Sharding: expert-parallel — core e holds expert e's three weight matrices.
Dispatch (gather of routed tokens) and combine (weighted scatter-add) happen on
the host as part of sharding/unsharding; each core runs a dense 3-matmul MLP on
its routed tokens (padded to a common capacity C) with the combine weight
applied on-device before writeback.

Matmul operands are fp16 (halves HBM traffic vs fp32, 1 cycle/row on the PE,
fast weight loads — unlike float32r which forces a ~190ns LDWEIGHTS per
matmul), accumulating in fp32 PSUM.  Values here are far inside fp16 range, and
fp16's 10-bit mantissa keeps the L2 relative error at ~5e-4.  All DMA feeds are
pre-laid-out on the host into the exact SBUF tile layouts so every DMA is
contiguous, and split into <=512KB pieces so they spread across DMA queues.
"""

import sys

sys.path.insert(0, "/opt/trn_rl_repo")

import numpy as np

import concourse.bass as bass
import concourse.mybir as mybir
import concourse.tile as tile
from concourse import bacc
from concourse.bass_utils import run_bass_kernel_spmd

# Fixed problem shape
T, D, FF, E, TOPK = 2048, 1024, 1536, 8, 2
P = 128
DSUB = D // P     # 8   k-subtiles over the D contraction
FBLK = FF // P    # 12  blocks over the FF dimension
NDN = 512         # free-dim tile for the down projection
NDT = D // NDN    # 2

F32 = mybir.dt.float32
F16 = mybir.dt.float16

_program_cache: dict[int, "bass.Bass"] = {}
last_results = None  # BassKernelResults of the most recent run (for profiling)


def _chunks(C: int) -> list[int]:
    """Split C (multiple of 128) into matmul moving-dim chunks of <=512
    (PSUM bank limit for fp32 accumulation)."""
    nb = C // P
    n = -(-nb * P // 512)  # ceil(C/512)
    base, rem = divmod(nb, n)
    return [(base + (1 if i < rem else 0)) * P for i in range(n)]


def _build_program(C: int) -> "bass.Bass":
    nc = bacc.Bacc(None, target_bir_lowering=False)

    xt_d = nc.dram_tensor("xt", [P, DSUB, C], F16, kind="ExternalInput")
    wg_d = nc.dram_tensor("wg", [FBLK, P, DSUB, P], F16, kind="ExternalInput")
    wu_d = nc.dram_tensor("wu", [FBLK, P, DSUB, P], F16, kind="ExternalInput")
    wd_d = nc.dram_tensor("wd", [FBLK, P, NDT, NDN], F16, kind="ExternalInput")
    wt_d = nc.dram_tensor("wt", [P, C // P], F32, kind="ExternalInput")
    y_d = nc.dram_tensor("y", [C // P, P, D], F32, kind="ExternalOutput")

    csizes = _chunks(C)

    with tile.TileContext(nc) as tc:
        with (
            tc.tile_pool(name="const", bufs=1) as const_pool,
            tc.tile_pool(name="wpool", bufs=3) as wpool,
            tc.tile_pool(name="actp", bufs=1) as act_pool,
            tc.tile_pool(name="sgp", bufs=3) as sg_pool,
            tc.tile_pool(name="yp", bufs=3) as y_pool,
            tc.tile_pool(name="psum", bufs=2, space="PSUM") as psum_pool,
            tc.tile_pool(name="psum_y", bufs=3, space="PSUM") as psum_y_pool,
            tc.tile_pool(name="psum_w", bufs=1, space="PSUM") as psum_w_pool,
        ):
            # HAM pre-warm: the PE clock sits at 1.2 GHz until it has been
            # busy for a ~3.4us activity window.  Chew through dummy matmuls
            # (no data dependencies) while the first DMAs are in flight so the
            # real matmuls start at 2.4 GHz.
            warm_in = const_pool.tile([P, NDN], F16)
            nc.vector.memset(warm_in[:], 0.0)
            warm_ps = psum_w_pool.tile([P, NDN], F32)
            for _ in range(8):
                nc.tensor.matmul(warm_ps[:], warm_in[:, :P], warm_in[:])

            # First weight block, then xt (split per k-subtile so the pieces
            # spread over DMA queues) — the first matmul group needs wg[0]
            # and xt[k=0], so get those in flight first.  All input DMAs go
            # through nc.sync (HWDGE); gpsimd's SWDGE dispatch is ~2x slower.
            wg_tiles = []
            wu_tiles = []
            wg_t = wpool.tile([P, DSUB, P], F16, tag="wg", name="wg0")
            nc.sync.dma_start(wg_t[:], wg_d[0])
            xt = const_pool.tile([P, DSUB, C], F16)
            for k in range(DSUB):
                nc.sync.dma_start(xt[:, k], xt_d[:, k])
            wu_t = wpool.tile([P, DSUB, P], F16, tag="wu", name="wu0")
            nc.sync.dma_start(wu_t[:], wu_d[0])
            wg_tiles.append(wg_t)
            wu_tiles.append(wu_t)
            wt_sb = const_pool.tile([P, C // P], F32)
            nc.sync.dma_start(wt_sb[:], wt_d[:])

            # wd lives in SBUF in full; its per-block loads are issued inside
            # the phase-1 loop so they fill otherwise-idle DMA time.
            wd_sb = const_pool.tile([P, FBLK, NDT, NDN], F16)

            # actT[f, fb, t] = silu(gateT) * upT, layout [128, FBLK, C]
            act = act_pool.tile([P, FBLK, C], F16)

            # Phase 1: gateT/upT = W @ hT per FF-block, fused silu*up
            for fb in range(FBLK):
                wg_t = wg_tiles[fb]
                wu_t = wu_tiles[fb]
                if fb + 1 < FBLK:
                    nwg = wpool.tile([P, DSUB, P], F16, tag="wg", name="wg")
                    nc.sync.dma_start(nwg[:], wg_d[fb + 1])
                    nwu = wpool.tile([P, DSUB, P], F16, tag="wu", name="wu")
                    nc.sync.dma_start(nwu[:], wu_d[fb + 1])
                    wg_tiles.append(nwg)
                    wu_tiles.append(nwu)
                # stream one wd block per fb iteration
                nc.gpsimd.dma_start(wd_sb[:, fb], wd_d[fb])

                col = 0
                for cs in csizes:
                    pg = psum_pool.tile([P, NDN], F32, tag="pg", name="pg")[:, :cs]
                    pu = psum_pool.tile([P, NDN], F32, tag="pu", name="pu")[:, :cs]
                    for k in range(DSUB):
                        nc.tensor.matmul(
                            pg,
                            wg_t[:, k, :],
                            xt[:, k, col : col + cs],
                            start=(k == 0),
                            stop=(k == DSUB - 1),
                        )
                    for k in range(DSUB):
                        nc.tensor.matmul(
                            pu,
                            wu_t[:, k, :],
                            xt[:, k, col : col + cs],
                            start=(k == 0),
                            stop=(k == DSUB - 1),
                        )
                    sg = sg_pool.tile([P, NDN], F32, tag="sg", name="sg")[:, :cs]
                    nc.scalar.activation(sg, pg, mybir.ActivationFunctionType.Silu)
                    nc.vector.tensor_mul(act[:, fb, col : col + cs], sg, pu)
                    col += cs

            # Phase 2: y[t, d] = (actT.T @ WdT) * combine_weight[t]
            for tb in range(C // P):
                for dti in range(NDT):
                    py = psum_y_pool.tile([P, NDN], F32, tag="py")
                    for fs in range(FBLK):
                        nc.tensor.matmul(
                            py,
                            act[:, fs, tb * P : (tb + 1) * P],
                            wd_sb[:, fs, dti, :],
                            start=(fs == 0),
                            stop=(fs == FBLK - 1),
                        )
                    y_sb = y_pool.tile([P, NDN], F32, tag="ysb")
                    nc.vector.tensor_scalar_mul(y_sb, py, wt_sb[:, tb : tb + 1])
                    nc.sync.dma_start(y_d[tb, :, dti * NDN : (dti + 1) * NDN], y_sb)

    nc.compile()
    return nc


def kernel(hidden_states, top_k_index, top_k_weights, gate_proj, up_proj, down_proj):
    global last_results

    h = np.ascontiguousarray(np.asarray(hidden_states, dtype=np.float32))
    idx = np.asarray(top_k_index)
    wts = np.asarray(top_k_weights, dtype=np.float32)
    gp = np.asarray(gate_proj, dtype=np.float32)
    up = np.asarray(up_proj, dtype=np.float32)
    dp = np.asarray(down_proj, dtype=np.float32)
    assert h.shape == (T, D) and idx.shape == (T, TOPK)
    assert gp.shape == (E, FF, D) and dp.shape == (E, D, FF)

    # combine[e, t] = sum_k wts[t, k] * (idx[t, k] == e)
    combine = np.zeros((E, T), np.float32)
    for k in range(TOPK):
        np.add.at(combine, (idx[:, k], np.arange(T)), wts[:, k])

    routed = [np.nonzero(combine[e] > 0)[0] for e in range(E)]
    max_cnt = max(len(r) for r in routed)
    C = max(P, -(-max_cnt // P) * P)

    if C not in _program_cache:
        _program_cache[C] = _build_program(C)
    nc = _program_cache[C]

    in_maps = []
    for e in range(E):
        r = routed[e]
        n_e = len(r)
        idx_pad = np.zeros(C, np.int64)
        idx_pad[:n_e] = r
        wt_pad = np.zeros(C, np.float32)
        wt_pad[:n_e] = combine[e, r]

        xg = h[idx_pad].astype(np.float16)  # [C, D]
        xt_feed = np.ascontiguousarray(xg.reshape(C, DSUB, P).transpose(2, 1, 0))
        wg_feed = np.ascontiguousarray(
            gp[e].astype(np.float16).reshape(FBLK, P, DSUB, P).transpose(0, 3, 2, 1)
        )
        wu_feed = np.ascontiguousarray(
            up[e].astype(np.float16).reshape(FBLK, P, DSUB, P).transpose(0, 3, 2, 1)
        )
        # wd_feed[fs, p, dt, dn] = down_proj[e][dt*NDN+dn, fs*P+p]
        wd_feed = np.ascontiguousarray(
            dp[e].astype(np.float16).reshape(NDT, NDN, FBLK, P).transpose(2, 3, 0, 1)
        )
        wt_feed = np.ascontiguousarray(wt_pad.reshape(C // P, P).T)
        in_maps.append(
            {"xt": xt_feed, "wg": wg_feed, "wu": wu_feed, "wd": wd_feed, "wt": wt_feed}
        )

    last_results = run_bass_kernel_spmd(nc, in_maps, core_ids=list(range(E)))

    out = np.zeros((T, D), np.float32)
    for e in range(E):
        r = routed[e]
        ye = last_results.results[e]["y"].reshape(C, D)
        out[r] += ye[: len(r)]
    return out


# revision 16
# speedup vs baseline: 1.1081x; 1.1081x over previous
"""MoE experts kernel for Trainium2 (Bass/Tile), expert-parallel across 8 NeuronCores.

Problem: nn_CompressedMoeExperts — T=2048 tokens, D=1024, FF=1536, E=8 experts,
top-k=2.  out[t] = sum_e combine[e,t] * (silu(h[t] @ Wg[e].T) * (h[t] @ Wu[e].T)) @ Wd[e].T

Sharding: expert-parallel — core e holds expert e's three weight matrices.
Dispatch (gather of routed tokens) and combine (weighted scatter-add) happen on
the host as part of sharding/unsharding; each core runs a dense 3-matmul MLP on
its routed tokens (padded to a common capacity C) with the combine weight
applied on-device before writeback.

Matmul operands are fp16 (halves HBM traffic vs fp32, 1 cycle/row on the PE,
fast weight loads — unlike float32r which forces a ~190ns LDWEIGHTS per
matmul), accumulating in fp32 PSUM.  Values here are far inside fp16 range, and
fp16's 10-bit mantissa keeps the L2 relative error at ~5e-4.  All DMA feeds are
pre-laid-out on the host into the exact SBUF tile layouts so every DMA is
contiguous, and split into <=512KB pieces so they spread across DMA queues.
"""

import os
import sys

sys.path.insert(0, "/opt/trn_rl_repo")

import numpy as np

import concourse.bass as bass
import concourse.mybir as mybir
import concourse.tile as tile
from concourse import bacc
from concourse.bass_utils import run_bass_kernel_spmd

# Fixed problem shape
T, D, FF, E, TOPK = 2048, 1024, 1536, 8, 2
P = 128
DSUB = D // P     # 8   k-subtiles over the D contraction
FBLK = FF // P    # 12  blocks over the FF dimension
NDN = 512         # free-dim tile for the down projection
NDT = D // NDN    # 2

F32 = mybir.dt.float32
F16 = mybir.dt.float16

_program_cache: dict[int, "bass.Bass"] = {}
last_results = None  # BassKernelResults of the most recent run (for profiling)


def _chunks(C: int) -> list[int]:
    """Split C (multiple of 128) into matmul moving-dim chunks of <=512
    (PSUM bank limit for fp32 accumulation)."""
    nb = C // P
    n = -(-nb * P // 512)  # ceil(C/512)
    base, rem = divmod(nb, n)
    return [(base + (1 if i < rem else 0)) * P for i in range(n)]


def _build_program(C: int) -> "bass.Bass":
    nc = bacc.Bacc(None, target_bir_lowering=False)

    xt_d = nc.dram_tensor("xt", [P, DSUB, C], F16, kind="ExternalInput")
    wg_d = nc.dram_tensor("wg", [FBLK, P, DSUB, P], F16, kind="ExternalInput")
    wu_d = nc.dram_tensor("wu", [FBLK, P, DSUB, P], F16, kind="ExternalInput")
    wd_d = nc.dram_tensor("wd", [FBLK, P, NDT, NDN], F16, kind="ExternalInput")
    wt_d = nc.dram_tensor("wt", [P, C // P], F32, kind="ExternalInput")
    y_d = nc.dram_tensor("y", [C // P, P, D], F32, kind="ExternalOutput")

    csizes = _chunks(C)

    with tile.TileContext(nc) as tc:
        with (
            tc.tile_pool(name="const", bufs=1) as const_pool,
            tc.tile_pool(name="wpool", bufs=3) as wpool,
            tc.tile_pool(name="actp", bufs=1) as act_pool,
            tc.tile_pool(name="sgp", bufs=3) as sg_pool,
            tc.tile_pool(name="yp", bufs=3) as y_pool,
            tc.tile_pool(name="psum", bufs=2, space="PSUM") as psum_pool,
            tc.tile_pool(name="psum_y", bufs=3, space="PSUM") as psum_y_pool,
            tc.tile_pool(name="psum_w", bufs=1, space="PSUM") as psum_w_pool,
        ):
            # HAM pre-warm: the PE clock sits at 1.2 GHz until it has been
            # busy for a ~3.4us activity window.  Chew through dummy matmuls
            # (no data dependencies) while the first DMAs are in flight so the
            # real matmuls start at 2.4 GHz.
            warm_in = const_pool.tile([P, NDN], F16)
            nc.vector.memset(warm_in[:], 0.0)
            warm_ps = psum_w_pool.tile([P, NDN], F32)
            for _ in range(10):
                nc.tensor.matmul(warm_ps[:], warm_in[:, :P], warm_in[:])

            # First weight block, then xt (split per k-subtile so the pieces
            # spread over DMA queues) — the first matmul group needs wg[0]
            # and xt[k=0], so get those in flight first.  All input DMAs go
            # through nc.sync (HWDGE); gpsimd's SWDGE dispatch is ~2x slower.
            wg_tiles = []
            wu_tiles = []
            wg_t = wpool.tile([P, DSUB, P], F16, tag="wg", name="wg0")
            nc.sync.dma_start(wg_t[:], wg_d[0])
            xt = const_pool.tile([P, DSUB, C], F16)
            for k in range(DSUB):
                nc.sync.dma_start(xt[:, k], xt_d[:, k])
            wu_t = wpool.tile([P, DSUB, P], F16, tag="wu", name="wu0")
            nc.sync.dma_start(wu_t[:], wu_d[0])
            wg_tiles.append(wg_t)
            wu_tiles.append(wu_t)
            wt_sb = const_pool.tile([P, C // P], F32)
            nc.sync.dma_start(wt_sb[:], wt_d[:])

            # wd lives in SBUF in full; its per-block loads are issued inside
            # the phase-1 loop so they fill otherwise-idle DMA time.
            wd_sb = const_pool.tile([P, FBLK, NDT, NDN], F16)

            # actT[f, fb, t] = silu(gateT) * upT, layout [128, FBLK, C]
            act = act_pool.tile([P, FBLK, C], F16)

            # Phase 1: gateT/upT = W @ hT per FF-block, fused silu*up
            for fb in range(FBLK):
                wg_t = wg_tiles[fb]
                wu_t = wu_tiles[fb]
                if fb + 1 < FBLK:
                    nwg = wpool.tile([P, DSUB, P], F16, tag="wg", name="wg")
                    nc.sync.dma_start(nwg[:], wg_d[fb + 1])
                    nwu = wpool.tile([P, DSUB, P], F16, tag="wu", name="wu")
                    nc.sync.dma_start(nwu[:], wu_d[fb + 1])
                    wg_tiles.append(nwg)
                    wu_tiles.append(nwu)
                # stream one wd block per fb iteration
                nc.sync.dma_start(wd_sb[:, fb], wd_d[fb])

                col = 0
                for cs in csizes:
                    pg = psum_pool.tile([P, NDN], F32, tag="pg", name="pg")[:, :cs]
                    pu = psum_pool.tile([P, NDN], F32, tag="pu", name="pu")[:, :cs]
                    for k in range(DSUB):
                        nc.tensor.matmul(
                            pg,
                            wg_t[:, k, :],
                            xt[:, k, col : col + cs],
                            start=(k == 0),
                            stop=(k == DSUB - 1),
                        )
                    for k in range(DSUB):
                        nc.tensor.matmul(
                            pu,
                            wu_t[:, k, :],
                            xt[:, k, col : col + cs],
                            start=(k == 0),
                            stop=(k == DSUB - 1),
                        )
                    sg = sg_pool.tile([P, NDN], F32, tag="sg", name="sg")[:, :cs]
                    nc.scalar.activation(sg, pg, mybir.ActivationFunctionType.Silu)
                    nc.vector.tensor_mul(act[:, fb, col : col + cs], sg, pu)
                    col += cs

            # Phase 2: y[t, d] = (actT.T @ WdT) * combine_weight[t]
            for tb in range(C // P):
                for dti in range(NDT):
                    py = psum_y_pool.tile([P, NDN], F32, tag="py")
                    for fs in range(FBLK):
                        nc.tensor.matmul(
                            py,
                            act[:, fs, tb * P : (tb + 1) * P],
                            wd_sb[:, fs, dti, :],
                            start=(fs == 0),
                            stop=(fs == FBLK - 1),
                        )
                    y_sb = y_pool.tile([P, NDN], F32, tag="ysb")
                    nc.vector.tensor_scalar_mul(y_sb, py, wt_sb[:, tb : tb + 1])
                    nc.sync.dma_start(y_d[tb, :, dti * NDN : (dti + 1) * NDN], y_sb)

    nc.compile()
    return nc


def kernel(hidden_states, top_k_index, top_k_weights, gate_proj, up_proj, down_proj):
    global last_results

    h = np.ascontiguousarray(np.asarray(hidden_states, dtype=np.float32))
    idx = np.asarray(top_k_index)
    wts = np.asarray(top_k_weights, dtype=np.float32)
    gp = np.asarray(gate_proj, dtype=np.float32)
    up = np.asarray(up_proj, dtype=np.float32)
    dp = np.asarray(down_proj, dtype=np.float32)
    assert h.shape == (T, D) and idx.shape == (T, TOPK)
    assert gp.shape == (E, FF, D) and dp.shape == (E, D, FF)

    # combine[e, t] = sum_k wts[t, k] * (idx[t, k] == e)
    combine = np.zeros((E, T), np.float32)
    for k in range(TOPK):
        np.add.at(combine, (idx[:, k], np.arange(T)), wts[:, k])

    routed = [np.nonzero(combine[e] > 0)[0] for e in range(E)]
    max_cnt = max(len(r) for r in routed)
    C = max(P, -(-max_cnt // P) * P)

    if C not in _program_cache:
        _program_cache[C] = _build_program(C)
    nc = _program_cache[C]

    in_maps = []
    for e in range(E):
        r = routed[e]
        n_e = len(r)
        idx_pad = np.zeros(C, np.int64)
        idx_pad[:n_e] = r
        wt_pad = np.zeros(C, np.float32)
        wt_pad[:n_e] = combine[e, r]

        xg = h[idx_pad].astype(np.float16)  # [C, D]
        xt_feed = np.ascontiguousarray(xg.reshape(C, DSUB, P).transpose(2, 1, 0))
        wg_feed = np.ascontiguousarray(
            gp[e].astype(np.float16).reshape(FBLK, P, DSUB, P).transpose(0, 3, 2, 1)
        )
        wu_feed = np.ascontiguousarray(
            up[e].astype(np.float16).reshape(FBLK, P, DSUB, P).transpose(0, 3, 2, 1)
        )
        # wd_feed[fs, p, dt, dn] = down_proj[e][dt*NDN+dn, fs*P+p]
        wd_feed = np.ascontiguousarray(
            dp[e].astype(np.float16).reshape(NDT, NDN, FBLK, P).transpose(2, 3, 0, 1)
        )
        wt_feed = np.ascontiguousarray(wt_pad.reshape(C // P, P).T)
        in_maps.append(
            {"xt": xt_feed, "wg": wg_feed, "wu": wu_feed, "wd": wd_feed, "wt": wt_feed}
        )

    ys = _run_on_device(C, in_maps)

    out = np.zeros((T, D), np.float32)
    for e in range(E):
        r = routed[e]
        ye = ys[e].reshape(C, D)
        out[r] += ye[: len(r)]
    return out


def _have_axon() -> bool:
    """The bass kernel executes via PJRT on the axon-tunneled NeuronCores.
    If the calling process pinned JAX_PLATFORMS=cpu (hiding them), fall back
    to a clean subprocess."""
    try:
        import jax

        return sum(1 for d in jax.devices() if getattr(d, "platform", "") != "cpu") >= E
    except Exception:
        return False


def _run_on_device(C: int, in_maps: list) -> list:
    global last_results
    if _have_axon():
        if C not in _program_cache:
            _program_cache[C] = _build_program(C)
        nc = _program_cache[C]
        last_results = run_bass_kernel_spmd(nc, in_maps, core_ids=list(range(E)))
        return [last_results.results[e]["y"] for e in range(E)]

    import pickle
    import subprocess
    import tempfile

    d = tempfile.mkdtemp()
    inp, outp = os.path.join(d, "in.pkl"), os.path.join(d, "out.pkl")
    with open(inp, "wb") as f:
        pickle.dump((C, in_maps), f)
    env = dict(os.environ)
    env.pop("JAX_PLATFORMS", None)
    subprocess.run(
        [sys.executable, os.path.abspath(__file__), "--device-run", inp, outp],
        check=True,
        env=env,
    )
    with open(outp, "rb") as f:
        return pickle.load(f)


if __name__ == "__main__" and "--device-run" in sys.argv:
    import pickle

    _inp, _outp = sys.argv[2], sys.argv[3]
    with open(_inp, "rb") as f:
        _C, _in_maps = pickle.load(f)
    _nc = _build_program(_C)
    _res = run_bass_kernel_spmd(_nc, _in_maps, core_ids=list(range(E)))
    with open(_outp, "wb") as f:
        pickle.dump([_res.results[e]["y"] for e in range(E)], f)


# revision 25
# speedup vs baseline: 1.1113x; 1.0030x over previous
"""MoE experts kernel for Trainium2 (Bass/Tile), expert-parallel across 8 NeuronCores.

Problem: nn_CompressedMoeExperts — T=2048 tokens, D=1024, FF=1536, E=8 experts,
top-k=2.  out[t] = sum_e combine[e,t] * (silu(h[t] @ Wg[e].T) * (h[t] @ Wu[e].T)) @ Wd[e].T

Sharding: expert-parallel — core e holds expert e's three weight matrices.
Dispatch (gather of routed tokens) and combine (weighted scatter-add) happen on
the host as part of sharding/unsharding; each core runs a dense 3-matmul MLP on
its routed tokens (padded to a common capacity C) with the combine weight
applied on-device before writeback.

Matmul operands are fp16 (halves HBM traffic vs fp32, 1 cycle/row on the PE,
fast weight loads — unlike float32r which forces a ~190ns LDWEIGHTS per
matmul), accumulating in fp32 PSUM.  Values here are far inside fp16 range, and
fp16's 10-bit mantissa keeps the L2 relative error at ~5e-4.  All DMA feeds are
pre-laid-out on the host into the exact SBUF tile layouts so every DMA is
contiguous, and split into <=512KB pieces so they spread across DMA queues.
"""

import os
import sys

sys.path.insert(0, "/opt/trn_rl_repo")

import numpy as np

import concourse.bass as bass
import concourse.mybir as mybir
import concourse.tile as tile
from concourse import bacc
from concourse.bass_utils import run_bass_kernel_spmd

# Fixed problem shape
T, D, FF, E, TOPK = 2048, 1024, 1536, 8, 2
P = 128
DSUB = D // P     # 8   k-subtiles over the D contraction
FBLK = FF // P    # 12  blocks over the FF dimension
NDN = 512         # free-dim tile for the down projection
NDT = D // NDN    # 2

F32 = mybir.dt.float32
F16 = mybir.dt.float16

_program_cache: dict[int, "bass.Bass"] = {}
last_results = None  # BassKernelResults of the most recent run (for profiling)


def _chunks(C: int) -> list[int]:
    """Split C (multiple of 128) into matmul moving-dim chunks of <=512
    (PSUM bank limit for fp32 accumulation)."""
    nb = C // P
    n = -(-nb * P // 512)  # ceil(C/512)
    base, rem = divmod(nb, n)
    return [(base + (1 if i < rem else 0)) * P for i in range(n)]


def _build_program(C: int) -> "bass.Bass":
    nc = bacc.Bacc(None, target_bir_lowering=False)

    xt_d = nc.dram_tensor("xt", [P, DSUB, C], F16, kind="ExternalInput")
    wg_d = nc.dram_tensor("wg", [FBLK, P, DSUB, P], F16, kind="ExternalInput")
    wu_d = nc.dram_tensor("wu", [FBLK, P, DSUB, P], F16, kind="ExternalInput")
    wd_d = nc.dram_tensor("wd", [FBLK, P, NDT, NDN], F16, kind="ExternalInput")
    wt_d = nc.dram_tensor("wt", [P, C // P], F32, kind="ExternalInput")
    y_d = nc.dram_tensor("y", [C // P, P, D], F32, kind="ExternalOutput")

    csizes = _chunks(C)

    with tile.TileContext(nc) as tc:
        with (
            tc.tile_pool(name="const", bufs=1) as const_pool,
            tc.tile_pool(name="wpool", bufs=3) as wpool,
            tc.tile_pool(name="actp", bufs=1) as act_pool,
            tc.tile_pool(name="sgp", bufs=3) as sg_pool,
            tc.tile_pool(name="yp", bufs=3) as y_pool,
            tc.tile_pool(name="psum", bufs=2, space="PSUM") as psum_pool,
            tc.tile_pool(name="psum_y", bufs=3, space="PSUM") as psum_y_pool,
            tc.tile_pool(name="psum_w", bufs=1, space="PSUM") as psum_w_pool,
        ):
            # HAM pre-warm: the PE clock sits at 1.2 GHz until it has been
            # busy for a ~3.4us activity window.  Chew through dummy matmuls
            # (no data dependencies) while the first DMAs are in flight so the
            # real matmuls start at 2.4 GHz.
            warm_in = const_pool.tile([P, NDN], F16)
            nc.vector.memset(warm_in[:], 0.0)
            warm_ps = psum_w_pool.tile([P, NDN], F32)
            for _ in range(10):
                nc.tensor.matmul(warm_ps[:], warm_in[:, :P], warm_in[:])

            # First weight block, then xt (split per k-subtile so the pieces
            # spread over DMA queues) — the first matmul group needs wg[0]
            # and xt[k=0], so get those in flight first.  All input DMAs go
            # through nc.sync (HWDGE); gpsimd's SWDGE dispatch is ~2x slower.
            wg_tiles = []
            wu_tiles = []
            wg_t = wpool.tile([P, DSUB, P], F16, tag="wg", name="wg0")
            nc.sync.dma_start(wg_t[:], wg_d[0])
            xt = const_pool.tile([P, DSUB, C], F16)
            for k in range(DSUB):
                nc.sync.dma_start(xt[:, k], xt_d[:, k])
            wu_t = wpool.tile([P, DSUB, P], F16, tag="wu", name="wu0")
            nc.sync.dma_start(wu_t[:], wu_d[0])
            wg_tiles.append(wg_t)
            wu_tiles.append(wu_t)
            wt_sb = const_pool.tile([P, C // P], F32)
            nc.sync.dma_start(wt_sb[:], wt_d[:])

            # wd lives in SBUF in full; its per-block loads are issued inside
            # the phase-1 loop so they fill otherwise-idle DMA time.
            wd_sb = const_pool.tile([P, FBLK, NDT, NDN], F16)

            # actT[f, fb, t] = silu(gateT) * upT, layout [128, FBLK, C]
            act = act_pool.tile([P, FBLK, C], F16)

            # Phase 1: gateT/upT = W @ hT per FF-block, fused silu*up
            for fb in range(FBLK):
                wg_t = wg_tiles[fb]
                wu_t = wu_tiles[fb]
                if fb + 1 < FBLK:
                    nwg = wpool.tile([P, DSUB, P], F16, tag="wg", name="wg")
                    nc.sync.dma_start(nwg[:], wg_d[fb + 1])
                    nwu = wpool.tile([P, DSUB, P], F16, tag="wu", name="wu")
                    nc.sync.dma_start(nwu[:], wu_d[fb + 1])
                    wg_tiles.append(nwg)
                    wu_tiles.append(nwu)
                # stream one wd block per fb iteration
                nc.sync.dma_start(wd_sb[:, fb], wd_d[fb])

                col = 0
                for cs in csizes:
                    pg = psum_pool.tile([P, NDN], F32, tag="pg", name="pg")[:, :cs]
                    pu = psum_pool.tile([P, NDN], F32, tag="pu", name="pu")[:, :cs]
                    for k in range(DSUB):
                        nc.tensor.matmul(
                            pg,
                            wg_t[:, k, :],
                            xt[:, k, col : col + cs],
                            start=(k == 0),
                            stop=(k == DSUB - 1),
                        )
                    for k in range(DSUB):
                        nc.tensor.matmul(
                            pu,
                            wu_t[:, k, :],
                            xt[:, k, col : col + cs],
                            start=(k == 0),
                            stop=(k == DSUB - 1),
                        )
                    sg = sg_pool.tile([P, NDN], F32, tag="sg", name="sg")[:, :cs]
                    nc.scalar.activation(sg, pg, mybir.ActivationFunctionType.Silu)
                    nc.vector.tensor_mul(act[:, fb, col : col + cs], sg, pu)
                    col += cs

            # Phase 2: y[t, d] = (actT.T @ WdT) * combine_weight[t]
            for tb in range(C // P):
                for dti in range(NDT):
                    py = psum_y_pool.tile([P, NDN], F32, tag="py")
                    for fs in range(FBLK):
                        nc.tensor.matmul(
                            py,
                            act[:, fs, tb * P : (tb + 1) * P],
                            wd_sb[:, fs, dti, :],
                            start=(fs == 0),
                            stop=(fs == FBLK - 1),
                        )
                    y_sb = y_pool.tile([P, NDN], F32, tag="ysb")
                    nc.vector.tensor_scalar_mul(y_sb, py, wt_sb[:, tb : tb + 1])
                    nc.sync.dma_start(y_d[tb, :, dti * NDN : (dti + 1) * NDN], y_sb)

    nc.compile()
    return nc


def _build_program_raw(C: int) -> "bass.Bass":
    """Hand-scheduled variant: same dataflow as _build_program but with manual
    per-engine streams and semaphores instead of TileContext — saves Tile's
    fixed ~6us startup barrier and ~9us tail drain-butterfly."""
    nc = bacc.Bacc(None, target_bir_lowering=False)

    xt_d = nc.dram_tensor("xt", [P, DSUB, C], F16, kind="ExternalInput")
    wg_d = nc.dram_tensor("wg", [FBLK, P, DSUB, P], F16, kind="ExternalInput")
    wu_d = nc.dram_tensor("wu", [FBLK, P, DSUB, P], F16, kind="ExternalInput")
    wd_d = nc.dram_tensor("wd", [FBLK, P, NDT, NDN], F16, kind="ExternalInput")
    wt_d = nc.dram_tensor("wt", [P, C // P], F32, kind="ExternalInput")
    y_d = nc.dram_tensor("y", [C // P, P, D], F32, kind="ExternalOutput")

    csizes = _chunks(C)
    NCH = len(csizes)
    G1 = FBLK * NCH          # gate groups (same count of up groups)
    NTB = C // P
    NY = NTB * NDT
    col0 = [sum(csizes[:i]) for i in range(NCH)]
    WARMN = 10

    from contextlib import ExitStack

    with ExitStack() as ctx:
        sb = lambda shape, dt, name: ctx.enter_context(nc.sbuf_tensor(name, shape, dt))
        ps = lambda name: ctx.enter_context(nc.psum_tensor(name, [P, NDN], F32))
        xt = sb([P, DSUB, C], F16, "xt_sb")
        wg_sb = sb([P, FBLK, DSUB, P], F16, "wg_sb")
        wu_sb = sb([P, FBLK, DSUB, P], F16, "wu_sb")
        wd_sb = sb([P, FBLK, NDT, NDN], F16, "wd_sb")
        act = sb([P, FBLK, C], F16, "act_sb")
        wt_sb = sb([P, C // P], F32, "wt_sb")
        warm_in = sb([P, NDN], F16, "warm_sb")
        sg = [sb([P, NDN], F32, f"sg{i}") for i in range(2)]
        y_sb = [sb([P, NDN], F32, f"ysb{i}") for i in range(3)]

        pg = [ps(f"pg{i}") for i in range(2)]
        pu = [ps(f"pu{i}") for i in range(2)]
        py = [ps(f"py{i}") for i in range(3)]
        warm_ps = ps("warm_ps")

        sem = lambda name: ctx.enter_context(nc.semaphore(name))
        sxt = [sem(f"sxt{k}") for k in range(DSUB)]
        swgu = [sem(f"swgu{fb}") for fb in range(FBLK)]
        swd = sem("swd")
        swt = sem("swt")
        swarm = sem("swarm")
        smm = sem("smm")
        sact = sem("sact")
        sdve = sem("sdve")
        sev = sem("sev")
        syd = sem("syd")

        with nc.Block() as block:

            @block.sync
            def _(sync):
                sync.dma_start(wg_sb[:, 0], wg_d[0]).then_inc(swgu[0], 16)
                sync.dma_start(wu_sb[:, 0], wu_d[0]).then_inc(swgu[0], 16)
                for k in range(DSUB):
                    sync.dma_start(xt[:, k], xt_d[:, k]).then_inc(sxt[k], 16)
                sync.dma_start(wt_sb[:], wt_d[:]).then_inc(swt, 16)
                for fb in range(1, FBLK):
                    sync.dma_start(wg_sb[:, fb], wg_d[fb]).then_inc(swgu[fb], 16)
                    sync.dma_start(wu_sb[:, fb], wu_d[fb]).then_inc(swgu[fb], 16)
                for fb in range(FBLK):
                    sync.dma_start(wd_sb[:, fb], wd_d[fb]).then_inc(swd, 16)
                for j in range(NY):
                    tb, dti = divmod(j, NDT)
                    sync.wait_ge(sev, j + 1)
                    sync.dma_start(
                        y_d[tb, :, dti * NDN : (dti + 1) * NDN], y_sb[j % 3][:]
                    ).then_inc(syd, 16)
                sync.wait_ge(syd, 16 * NY)

            @block.tensor
            def _(tensor):
                tensor.wait_ge(swarm, 1)
                for _ in range(WARMN):
                    nc.tensor.matmul(warm_ps[:], warm_in[:, :P], warm_in[:])
                g = 0
                for fb in range(FBLK):
                    tensor.wait_ge(swgu[fb], 32)
                    for c in range(NCH):
                        cs, c0 = csizes[c], col0[c]
                        if g >= 2:
                            tensor.wait_ge(sact, g - 1)
                        for k in range(DSUB):
                            if g == 0:
                                tensor.wait_ge(sxt[k], 16)
                            mm = nc.tensor.matmul(
                                pg[g % 2][:, :cs],
                                wg_sb[:, fb, k, :],
                                xt[:, k, c0 : c0 + cs],
                                start=(k == 0),
                                stop=(k == DSUB - 1),
                            )
                        mm.then_inc(smm, 1)
                        if g >= 2:
                            tensor.wait_ge(sdve, g - 1)
                        for k in range(DSUB):
                            mm = nc.tensor.matmul(
                                pu[g % 2][:, :cs],
                                wu_sb[:, fb, k, :],
                                xt[:, k, c0 : c0 + cs],
                                start=(k == 0),
                                stop=(k == DSUB - 1),
                            )
                        mm.then_inc(smm, 1)
                        g += 1
                tensor.wait_ge(sdve, G1)
                tensor.wait_ge(swd, 16 * FBLK)
                for j in range(NY):
                    tb, dti = divmod(j, NDT)
                    if j >= 3:
                        tensor.wait_ge(sev, j - 2)
                    for fs in range(FBLK):
                        mm = nc.tensor.matmul(
                            py[j % 3][:],
                            act[:, fs, tb * P : (tb + 1) * P],
                            wd_sb[:, fs, dti, :],
                            start=(fs == 0),
                            stop=(fs == FBLK - 1),
                        )
                    mm.then_inc(smm, 1)

            @block.scalar
            def _(scalar):
                for g in range(G1):
                    cs = csizes[g % NCH]
                    if g >= 2:
                        scalar.wait_ge(sdve, g - 1)
                    scalar.wait_ge(smm, 2 * g + 1)
                    nc.scalar.activation(
                        sg[g % 2][:, :cs],
                        pg[g % 2][:, :cs],
                        mybir.ActivationFunctionType.Silu,
                    ).then_inc(sact, 1)

            @block.vector
            def _(vector):
                nc.vector.memset(warm_in[:], 0.0).then_inc(swarm, 1)
                for g in range(G1):
                    fb, c = divmod(g, NCH)
                    cs, c0 = csizes[c], col0[c]
                    vector.wait_ge(sact, g + 1)
                    vector.wait_ge(smm, 2 * g + 2)
                    nc.vector.tensor_mul(
                        act[:, fb, c0 : c0 + cs], sg[g % 2][:, :cs], pu[g % 2][:, :cs]
                    ).then_inc(sdve, 1)
                vector.wait_ge(swt, 16)
                for j in range(NY):
                    tb = j // NDT
                    if j >= 3:
                        # y DMAs can complete out of order across HW queues, so
                        # wait for ALL previously dispatched ones before
                        # reusing the j%3 staging buffer.
                        vector.wait_ge(syd, 16 * j)
                    vector.wait_ge(smm, 2 * G1 + j + 1)
                    nc.vector.tensor_scalar_mul(
                        y_sb[j % 3][:], py[j % 3][:], wt_sb[:, tb : tb + 1]
                    ).then_inc(sev, 1)

        nc.compile()
    return nc


_RAW = os.environ.get("KERNEL_RAW", "0") == "1"


def kernel(hidden_states, top_k_index, top_k_weights, gate_proj, up_proj, down_proj):
    global last_results

    h = np.ascontiguousarray(np.asarray(hidden_states, dtype=np.float32))
    idx = np.asarray(top_k_index)
    wts = np.asarray(top_k_weights, dtype=np.float32)
    gp = np.asarray(gate_proj, dtype=np.float32)
    up = np.asarray(up_proj, dtype=np.float32)
    dp = np.asarray(down_proj, dtype=np.float32)
    assert h.shape == (T, D) and idx.shape == (T, TOPK)
    assert gp.shape == (E, FF, D) and dp.shape == (E, D, FF)

    # combine[e, t] = sum_k wts[t, k] * (idx[t, k] == e)
    combine = np.zeros((E, T), np.float32)
    for k in range(TOPK):
        np.add.at(combine, (idx[:, k], np.arange(T)), wts[:, k])

    routed = [np.nonzero(combine[e] > 0)[0] for e in range(E)]
    max_cnt = max(len(r) for r in routed)
    C = max(P, -(-max_cnt // P) * P)

    if C not in _program_cache:
        _program_cache[C] = _build_program(C)
    nc = _program_cache[C]

    in_maps = []
    for e in range(E):
        r = routed[e]
        n_e = len(r)
        idx_pad = np.zeros(C, np.int64)
        idx_pad[:n_e] = r
        wt_pad = np.zeros(C, np.float32)
        wt_pad[:n_e] = combine[e, r]

        xg = h[idx_pad].astype(np.float16)  # [C, D]
        xt_feed = np.ascontiguousarray(xg.reshape(C, DSUB, P).transpose(2, 1, 0))
        wg_feed = np.ascontiguousarray(
            gp[e].astype(np.float16).reshape(FBLK, P, DSUB, P).transpose(0, 3, 2, 1)
        )
        wu_feed = np.ascontiguousarray(
            up[e].astype(np.float16).reshape(FBLK, P, DSUB, P).transpose(0, 3, 2, 1)
        )
        # wd_feed[fs, p, dt, dn] = down_proj[e][dt*NDN+dn, fs*P+p]
        wd_feed = np.ascontiguousarray(
            dp[e].astype(np.float16).reshape(NDT, NDN, FBLK, P).transpose(2, 3, 0, 1)
        )
        wt_feed = np.ascontiguousarray(wt_pad.reshape(C // P, P).T)
        in_maps.append(
            {"xt": xt_feed, "wg": wg_feed, "wu": wu_feed, "wd": wd_feed, "wt": wt_feed}
        )

    ys = _run_on_device(C, in_maps)

    out = np.zeros((T, D), np.float32)
    for e in range(E):
        r = routed[e]
        ye = ys[e].reshape(C, D)
        out[r] += ye[: len(r)]
    return out


def _have_axon() -> bool:
    """The bass kernel executes via PJRT on the axon-tunneled NeuronCores.
    If the calling process pinned JAX_PLATFORMS=cpu (hiding them), fall back
    to a clean subprocess."""
    try:
        import jax

        return sum(1 for d in jax.devices() if getattr(d, "platform", "") != "cpu") >= E
    except Exception:
        return False


def _run_on_device(C: int, in_maps: list) -> list:
    global last_results
    if _have_axon():
        if C not in _program_cache:
            _program_cache[C] = (
                _build_program_raw(C) if _RAW else _build_program(C)
            )
        nc = _program_cache[C]
        last_results = run_bass_kernel_spmd(nc, in_maps, core_ids=list(range(E)))
        return [last_results.results[e]["y"] for e in range(E)]

    import pickle
    import subprocess
    import tempfile

    d = tempfile.mkdtemp()
    inp, outp = os.path.join(d, "in.pkl"), os.path.join(d, "out.pkl")
    with open(inp, "wb") as f:
        pickle.dump((C, in_maps), f)
    env = dict(os.environ)
    env.pop("JAX_PLATFORMS", None)
    subprocess.run(
        [sys.executable, os.path.abspath(__file__), "--device-run", inp, outp],
        check=True,
        env=env,
    )
    with open(outp, "rb") as f:
        return pickle.load(f)


if __name__ == "__main__" and "--device-run" in sys.argv:
    import pickle

    _inp, _outp = sys.argv[2], sys.argv[3]
    with open(_inp, "rb") as f:
        _C, _in_maps = pickle.load(f)
    _nc = _build_program_raw(_C) if _RAW else _build_program(_C)
    _res = run_bass_kernel_spmd(_nc, _in_maps, core_ids=list(range(E)))
    with open(_outp, "wb") as f:
        pickle.dump([_res.results[e]["y"] for e in range(E)], f)


# revision 26
# speedup vs baseline: 1.2013x; 1.0810x over previous
"""MoE experts kernel for Trainium2 (Bass/Tile), expert-parallel across 8 NeuronCores.

Problem: nn_CompressedMoeExperts — T=2048 tokens, D=1024, FF=1536, E=8 experts,
top-k=2.  out[t] = sum_e combine[e,t] * (silu(h[t] @ Wg[e].T) * (h[t] @ Wu[e].T)) @ Wd[e].T

Sharding: expert-parallel with FF-split load balancing.  Each expert's MLP is
split into two independent shards along the FF dimension (rows of Wg/Wu,
columns of Wd — their partial down-projection outputs simply add).  The 16
shards are sorted by routed-token count and dealt out so every core gets one
"big" and one "small" shard; per-core compute is then proportional to
C0+C1 (max big + max small capacity) instead of 2*C_max, which removes the
hot-expert straggler penalty under skewed routing.  Token dispatch (gather by
top_k_index) and the weighted combine scatter-add happen on the host as part
of sharding/unsharding; the combine weight itself is applied on-device.

Matmul operands are fp16 (halves HBM traffic vs fp32, 1 cycle/row on the PE,
fast weight loads — unlike float32r which forces a ~190ns LDWEIGHTS reload per
matmul), accumulating in fp32 PSUM.  Values are far inside fp16 range and the
10-bit mantissa keeps L2 relative error ~5e-4.  All DMA feeds are pre-laid-out
on the host into exact SBUF tile layouts so every DMA is contiguous, split into
~0.5MB pieces to spread across DMA queues.  A short run of dummy matmuls warms
the PE clock (HAM gate: 1.2 GHz until ~3.4us of sustained activity) while the
first DMAs stage.
"""

import os
import sys

sys.path.insert(0, "/opt/trn_rl_repo")

import numpy as np

import concourse.bass as bass
import concourse.mybir as mybir
import concourse.tile as tile
from concourse import bacc
from concourse.bass_utils import run_bass_kernel_spmd

# Fixed problem shape
T, D, FF, E, TOPK = 2048, 1024, 1536, 8, 2
P = 128
DSUB = D // P     # 8   k-subtiles over the D contraction
FBLK = FF // P    # 12  128-row blocks over the full FF dimension
FBH = FBLK // 2   # 6   blocks per FF-half shard
FH = FF // 2      # 768
NDN = 512         # free-dim tile for the down projection
NDT = D // NDN    # 2

F32 = mybir.dt.float32
F16 = mybir.dt.float16

_program_cache: dict[tuple, "bass.Bass"] = {}
last_results = None  # BassKernelResults of the most recent run (for profiling)


def _chunks(C: int) -> list[int]:
    """Split C (multiple of 128) into matmul moving-dim chunks of <=512
    (PSUM bank limit for fp32 accumulation)."""
    nb = C // P
    n = -(-nb * P // 512)  # ceil(C/512)
    base, rem = divmod(nb, n)
    return [(base + (1 if i < rem else 0)) * P for i in range(n)]


def _build_program(C0: int, C1: int) -> "bass.Bass":
    nc = bacc.Bacc(None, target_bir_lowering=False)

    Cs = (C0, C1)
    xt_d = [
        nc.dram_tensor(f"xt{s}", [P, DSUB, Cs[s]], F16, kind="ExternalInput")
        for s in range(2)
    ]
    wg_d = nc.dram_tensor("wg", [FBLK, P, DSUB, P], F16, kind="ExternalInput")
    wu_d = nc.dram_tensor("wu", [FBLK, P, DSUB, P], F16, kind="ExternalInput")
    wd_d = nc.dram_tensor("wd", [FBLK, P, NDT, NDN], F16, kind="ExternalInput")
    wt_d = [
        nc.dram_tensor(f"wt{s}", [P, Cs[s] // P], F32, kind="ExternalInput")
        for s in range(2)
    ]
    y_d = [
        nc.dram_tensor(f"y{s}", [Cs[s] // P, P, D], F32, kind="ExternalOutput")
        for s in range(2)
    ]

    with tile.TileContext(nc) as tc:
        with (
            tc.tile_pool(name="const", bufs=1) as const_pool,
            tc.tile_pool(name="wpool", bufs=3) as wpool,
            tc.tile_pool(name="actp", bufs=1) as act_pool,
            tc.tile_pool(name="sgp", bufs=3) as sg_pool,
            tc.tile_pool(name="yp", bufs=3) as y_pool,
            tc.tile_pool(name="psum", bufs=2, space="PSUM") as psum_pool,
            tc.tile_pool(name="psum_y", bufs=3, space="PSUM") as psum_y_pool,
            tc.tile_pool(name="psum_w", bufs=1, space="PSUM") as psum_w_pool,
        ):
            # HAM pre-warm: dummy matmuls (only dep: the memset) keep the PE
            # busy while the first DMAs stage, so real matmuls start at 2.4GHz.
            warm_in = const_pool.tile([P, NDN], F16)
            nc.vector.memset(warm_in[:], 0.0)
            warm_ps = psum_w_pool.tile([P, NDN], F32)
            for _ in range(10):
                nc.tensor.matmul(warm_ps[:], warm_in[:, :P], warm_in[:])

            # First weight block, then shard0's tokens (split per k-subtile to
            # spread over DMA queues), then everything else.  All input DMAs on
            # nc.sync (HWDGE): gpsimd SWDGE dispatch is ~2x slower.
            wg_tiles = {}
            wu_tiles = {}
            wg_tiles[0] = wpool.tile([P, DSUB, P], F16, tag="wg", name="wg0")
            nc.sync.dma_start(wg_tiles[0][:], wg_d[0])
            xt = []
            xt.append(const_pool.tile([P, DSUB, C0], F16, name="xt0"))
            for k in range(DSUB):
                nc.sync.dma_start(xt[0][:, k], xt_d[0][:, k])
            wu_tiles[0] = wpool.tile([P, DSUB, P], F16, tag="wu", name="wu0")
            nc.sync.dma_start(wu_tiles[0][:], wu_d[0])
            wt_sb = [const_pool.tile([P, Cs[0] // P], F32, name="wt0")]
            nc.sync.dma_start(wt_sb[0][:], wt_d[0][:])

            # shard1 token feed + combine weights, needed only after shard0
            xt.append(const_pool.tile([P, DSUB, C1], F16, name="xt1"))
            wt_sb.append(const_pool.tile([P, Cs[1] // P], F32, name="wt1"))

            wd_sb = const_pool.tile([P, FBLK, NDT, NDN], F16)

            act = [
                act_pool.tile([P, FBH, C0], F16, name="act0"),
                act_pool.tile([P, FBH, C1], F16, name="act1"),
            ]

            for s in range(2):
                C = Cs[s]
                csizes = _chunks(C)
                if s == 1:
                    for k in range(DSUB):
                        nc.sync.dma_start(xt[1][:, k], xt_d[1][:, k])
                    nc.sync.dma_start(wt_sb[1][:], wt_d[1][:])

                # Phase 1: gateT/upT per FF-block of this shard, fused silu*up
                for fbl in range(FBH):
                    fb = s * FBH + fbl
                    wg_t = wg_tiles.pop(fb)
                    wu_t = wu_tiles.pop(fb)
                    if fb + 1 < FBLK:
                        nwg = wpool.tile([P, DSUB, P], F16, tag="wg", name="wg")
                        nc.sync.dma_start(nwg[:], wg_d[fb + 1])
                        nwu = wpool.tile([P, DSUB, P], F16, tag="wu", name="wu")
                        nc.sync.dma_start(nwu[:], wu_d[fb + 1])
                        wg_tiles[fb + 1] = nwg
                        wu_tiles[fb + 1] = nwu
                    # stream one wd block per iteration
                    nc.sync.dma_start(wd_sb[:, fb], wd_d[fb])

                    col = 0
                    for cs in csizes:
                        pg = psum_pool.tile([P, NDN], F32, tag="pg", name="pg")[:, :cs]
                        pu = psum_pool.tile([P, NDN], F32, tag="pu", name="pu")[:, :cs]
                        for k in range(DSUB):
                            nc.tensor.matmul(
                                pg,
                                wg_t[:, k, :],
                                xt[s][:, k, col : col + cs],
                                start=(k == 0),
                                stop=(k == DSUB - 1),
                            )
                        for k in range(DSUB):
                            nc.tensor.matmul(
                                pu,
                                wu_t[:, k, :],
                                xt[s][:, k, col : col + cs],
                                start=(k == 0),
                                stop=(k == DSUB - 1),
                            )
                        sg = sg_pool.tile([P, NDN], F32, tag="sg", name="sg")[:, :cs]
                        nc.scalar.activation(
                            sg, pg, mybir.ActivationFunctionType.Silu
                        )
                        nc.vector.tensor_mul(act[s][:, fbl, col : col + cs], sg, pu)
                        col += cs

                # Phase 2: y[t, d] = (actT.T @ WdT_half) * combine_weight[t]
                for tb in range(C // P):
                    for dti in range(NDT):
                        py = psum_y_pool.tile([P, NDN], F32, tag="py")
                        for fs in range(FBH):
                            nc.tensor.matmul(
                                py,
                                act[s][:, fs, tb * P : (tb + 1) * P],
                                wd_sb[:, s * FBH + fs, dti, :],
                                start=(fs == 0),
                                stop=(fs == FBH - 1),
                            )
                        y_sb = y_pool.tile([P, NDN], F32, tag="ysb")
                        nc.vector.tensor_scalar_mul(
                            y_sb, py, wt_sb[s][:, tb : tb + 1]
                        )
                        nc.sync.dma_start(
                            y_d[s][tb, :, dti * NDN : (dti + 1) * NDN], y_sb
                        )

    nc.compile()
    return nc


def _shard_feed(h16, gp, up, dp, combine, routed, e, half, C):
    """Build one (expert, FF-half) shard's DMA feeds, pre-laid-out to match the
    kernel's SBUF tile layouts exactly (every DMA contiguous)."""
    r = routed[e]
    n_e = len(r)
    idx_pad = np.zeros(C, np.int64)
    idx_pad[:n_e] = r
    wt_pad = np.zeros(C, np.float32)
    wt_pad[:n_e] = combine[e, r]
    hs = slice(half * FH, (half + 1) * FH)

    xg = h16[idx_pad]  # [C, D] fp16
    xt_feed = np.ascontiguousarray(xg.reshape(C, DSUB, P).transpose(2, 1, 0))
    wg_feed = np.ascontiguousarray(
        gp[e][hs, :].astype(np.float16).reshape(FBH, P, DSUB, P).transpose(0, 3, 2, 1)
    )
    wu_feed = np.ascontiguousarray(
        up[e][hs, :].astype(np.float16).reshape(FBH, P, DSUB, P).transpose(0, 3, 2, 1)
    )
    # wd_feed[fs, p, dt, dn] = down_proj[e][dt*NDN+dn, half*FH + fs*P+p]
    wd_feed = np.ascontiguousarray(
        dp[e][:, hs].astype(np.float16).reshape(NDT, NDN, FBH, P).transpose(2, 3, 0, 1)
    )
    wt_feed = np.ascontiguousarray(wt_pad.reshape(C // P, P).T)
    return xt_feed, wg_feed, wu_feed, wd_feed, wt_feed


def kernel(hidden_states, top_k_index, top_k_weights, gate_proj, up_proj, down_proj):
    h = np.ascontiguousarray(np.asarray(hidden_states, dtype=np.float32))
    idx = np.asarray(top_k_index)
    wts = np.asarray(top_k_weights, dtype=np.float32)
    gp = np.asarray(gate_proj, dtype=np.float32)
    up = np.asarray(up_proj, dtype=np.float32)
    dp = np.asarray(down_proj, dtype=np.float32)
    assert h.shape == (T, D) and idx.shape == (T, TOPK)
    assert gp.shape == (E, FF, D) and dp.shape == (E, D, FF)

    # combine[e, t] = sum_k wts[t, k] * (idx[t, k] == e)
    combine = np.zeros((E, T), np.float32)
    for k in range(TOPK):
        np.add.at(combine, (idx[:, k], np.arange(T)), wts[:, k])

    routed = [np.nonzero(combine[e] > 0)[0] for e in range(E)]
    cnt = [len(r) for r in routed]

    # 16 (expert, FF-half) shards; 8 largest go in slot0, 8 smallest in slot1.
    shards = sorted(
        ((e, half) for e in range(E) for half in range(2)),
        key=lambda s: -cnt[s[0]],
    )
    slot0, slot1 = shards[:E], shards[E:]
    pad = lambda n: max(P, -(-n // P) * P)
    C0 = pad(max(cnt[e] for e, _ in slot0))
    C1 = pad(max(cnt[e] for e, _ in slot1))

    h16 = h.astype(np.float16)
    in_maps = []
    for core in range(E):
        m = {}
        wg_parts, wu_parts, wd_parts = [], [], []
        for s, ((e, half), C) in enumerate([(slot0[core], C0), (slot1[core], C1)]):
            xt_f, wg_f, wu_f, wd_f, wt_f = _shard_feed(
                h16, gp, up, dp, combine, routed, e, half, C
            )
            m[f"xt{s}"] = xt_f
            m[f"wt{s}"] = wt_f
            wg_parts.append(wg_f)
            wu_parts.append(wu_f)
            wd_parts.append(wd_f)
        m["wg"] = np.ascontiguousarray(np.concatenate(wg_parts, axis=0))
        m["wu"] = np.ascontiguousarray(np.concatenate(wu_parts, axis=0))
        m["wd"] = np.ascontiguousarray(np.concatenate(wd_parts, axis=0))
        in_maps.append(m)

    ys = _run_on_device(C0, C1, in_maps)

    out = np.zeros((T, D), np.float32)
    for core in range(E):
        for s, ((e, half), C) in enumerate([(slot0[core], C0), (slot1[core], C1)]):
            r = routed[e]
            out[r] += ys[core][s].reshape(C, D)[: len(r)]
    return out


def _have_axon() -> bool:
    """The bass kernel executes via PJRT on the axon-tunneled NeuronCores.
    If the calling process pinned JAX_PLATFORMS=cpu (hiding them), fall back
    to a clean subprocess."""
    try:
        import jax

        return sum(1 for d in jax.devices() if getattr(d, "platform", "") != "cpu") >= E
    except Exception:
        return False


def _run_on_device(C0: int, C1: int, in_maps: list) -> list:
    global last_results
    if _have_axon():
        key = (C0, C1)
        if key not in _program_cache:
            _program_cache[key] = _build_program(C0, C1)
        nc = _program_cache[key]
        last_results = run_bass_kernel_spmd(nc, in_maps, core_ids=list(range(E)))
        return [
            (last_results.results[c]["y0"], last_results.results[c]["y1"])
            for c in range(E)
        ]

    import pickle
    import subprocess
    import tempfile

    d = tempfile.mkdtemp()
    inp, outp = os.path.join(d, "in.pkl"), os.path.join(d, "out.pkl")
    with open(inp, "wb") as f:
        pickle.dump((C0, C1, in_maps), f)
    env = dict(os.environ)
    env.pop("JAX_PLATFORMS", None)
    subprocess.run(
        [sys.executable, os.path.abspath(__file__), "--device-run", inp, outp],
        check=True,
        env=env,
    )
    with open(outp, "rb") as f:
        return pickle.load(f)


if __name__ == "__main__" and "--device-run" in sys.argv:
    import pickle

    _inp, _outp = sys.argv[2], sys.argv[3]
    with open(_inp, "rb") as f:
        _C0, _C1, _in_maps = pickle.load(f)
    _nc = _build_program(_C0, _C1)
    _res = run_bass_kernel_spmd(_nc, _in_maps, core_ids=list(range(E)))
    with open(_outp, "wb") as f:
        pickle.dump(
            [(_res.results[c]["y0"], _res.results[c]["y1"]) for c in range(E)], f
        )
